# revision 1
# baseline (speedup 1.0000x reference)
"""BarrierNet forward on 8 Trainium2 NeuronCores (pure batch data-parallel).

Math actually needed (x32 / x0 branches of the reference are dead code):
    h   = relu(x @ W1 + b1)                       [B, 2048]
    a   = relu(h @ W21 + b21)                     [B, 1024]
    t   = a @ W31                                 [B, 2]    (bias folded below)
    out = clip(-t + bias2, lo2, hi2)              [B, 2]
with host-folded per-channel constants
    bias2 = -(b31 + 2*om/os),  lo2 = (lo-om)/os,  hi2 = (hi-om)/os
    lo = [-(1+s3), -(1+s1)],   hi = [1+s2, 1+s0]

Device dataflow keeps features on the partition dim (x^T -> h^T -> a^T ->
x31^T) so every weight matrix is used directly as the stationary lhsT and
only the tiny x / out tensors ever need a transpose (done on host / via a
strided DMA).
"""

import os

import numpy as np

B, N_IN, H1, H2, N_CL = 32768, 8, 2048, 1024, 2
N_CORES = 8
B_SH = B // N_CORES  # 4096 rows per core
NB = 512             # batch-chunk width (matmul free dim / PSUM bank)
N_CHUNKS = B_SH // NB
MT1 = H1 // 128      # 16 output tiles of mm1
KT2, MT2 = H1 // 128, H2 // 128  # 16 k-tiles, 8 m-tiles of mm2
KT3 = H2 // 128      # 8 k-tiles of mm3

MM_MODE = os.environ.get("BARRIER_MM_MODE", "fp32r")  # fp32r | bf16 | fp32
TRACE = bool(int(os.environ.get("BARRIER_TRACE", "0")))

_CACHE = {}
last_results = None  # BassKernelResults of the most recent run (for test.py)


def _build(mode):
    from contextlib import ExitStack

    import concourse.bass as bass
    import concourse.mybir as mybir
    import concourse.tile as tile
    from concourse import bacc

    f32 = mybir.dt.float32
    if mode == "bf16":
        io_dt = mybir.dt.bfloat16
    elif mode == "fp32r":
        io_dt = mybir.dt.float32r
    else:
        io_dt = f32

    def mm(ap):
        return ap

    nc = bacc.Bacc("TRN2", debug=False, num_devices=N_CORES)

    xT_d = nc.dram_tensor("xT", [N_IN, B_SH], io_dt, kind="ExternalInput").ap()
    w1_d = nc.dram_tensor("w1", [N_IN, H1], io_dt, kind="ExternalInput").ap()
    w21_d = nc.dram_tensor("w21", [H1, H2], io_dt, kind="ExternalInput").ap()
    w31_d = nc.dram_tensor("w31", [H2, N_CL], io_dt, kind="ExternalInput").ap()
    b1_d = nc.dram_tensor("b1", [H1], f32, kind="ExternalInput").ap()
    b21_d = nc.dram_tensor("b21", [H2], f32, kind="ExternalInput").ap()
    post_d = nc.dram_tensor("post", [N_CL, 3], f32, kind="ExternalInput").ap()
    out_d = nc.dram_tensor("out", [B_SH, N_CL], f32, kind="ExternalOutput").ap()

    Relu = mybir.ActivationFunctionType.Relu
    Ident = mybir.ActivationFunctionType.Identity
    add_op = mybir.AluOpType.add
    max_op = mybir.AluOpType.max
    min_op = mybir.AluOpType.min

    with tile.TileContext(nc) as tc, ExitStack() as ctx:
        const = ctx.enter_context(tc.tile_pool(name="const", bufs=1))
        wpool = ctx.enter_context(tc.tile_pool(name="w21", bufs=1))
        hpool = ctx.enter_context(tc.tile_pool(name="hT", bufs=3))
        apool = ctx.enter_context(tc.tile_pool(name="aT", bufs=1))
        opool = ctx.enter_context(tc.tile_pool(name="post", bufs=2))
        ps_h = ctx.enter_context(tc.tile_pool(name="ps_h", bufs=4, space="PSUM"))
        ps_a = ctx.enter_context(tc.tile_pool(name="ps_a", bufs=3, space="PSUM"))
        ps_o = ctx.enter_context(tc.tile_pool(name="ps_o", bufs=1, space="PSUM"))

        # Stationary weights / constants. w1 and xT are replicated into all
        # four 32-row groups of the PE array so four K=8 matmuls can run
        # concurrently via tile_position.
        # DMA issue order is the critical path: SP issues serially (~0.6us
        # each) and nothing lands before ~12us of NEFF startup. Put exactly
        # what the first matmuls + evacs need first: row-group-0 of xT/w1,
        # then b1; remaining row groups and constants; then the 8MB W21.
        # xT / w1 are replicated into the four 32-row groups of the PE array
        # with four direct DRAM->SBUF DMAs each (disjoint partition bands).
        w1_sb = const.tile([128, H1], io_dt)
        xT_sb = const.tile([128, B_SH], io_dt)
        nc.sync.dma_start(out=xT_sb[0:N_IN, :], in_=xT_d)
        nc.sync.dma_start(out=w1_sb[0:N_IN, :], in_=w1_d)
        b1_sb = const.tile([128, MT1], f32)
        nc.sync.dma_start(out=b1_sb, in_=b1_d.rearrange("(k p) -> p k", p=128))
        b21_sb = const.tile([128, MT2], f32)
        nc.sync.dma_start(out=b21_sb, in_=b21_d.rearrange("(k p) -> p k", p=128))
        post_sb = const.tile([N_CL, 3], f32)
        nc.sync.dma_start(out=post_sb, in_=post_d)
        for g in range(1, 4):
            nc.sync.dma_start(out=xT_sb[32 * g : 32 * g + N_IN, :], in_=xT_d)
            nc.sync.dma_start(out=w1_sb[32 * g : 32 * g + N_IN, :], in_=w1_d)
        w31_sb = const.tile([128, KT3 * N_CL], io_dt)
        for k in range(KT3):
            nc.sync.dma_start(
                out=w31_sb[:, k * N_CL : (k + 1) * N_CL],
                in_=w31_d[k * 128 : (k + 1) * 128, :],
            )
        w21_t = []
        for k in range(KT2):
            t = wpool.tile([128, H2], io_dt, tag=f"w21_{k}")
            nc.sync.dma_start(out=t, in_=w21_d[k * 128 : (k + 1) * 128, :])
            w21_t.append(t)

        def mm1(c):
            # hT = relu(W1^T @ xT + b1), K=8, 4-way row-group packed.
            # Chunk 0 walks row groups in DMA-arrival order (all of group 0
            # first) so its first matmuls only wait on the band-0 loads.
            bs = slice(c * NB, (c + 1) * NB)
            hT = [None] * MT1
            order = (
                [4 * j + g for g in range(4) for j in range(4)]
                if c == 0
                else range(MT1)
            )
            for m in order:
                g = m % 4
                ph = ps_h.tile([128, NB], f32)
                nc.tensor.matmul(
                    ph,
                    mm(w1_sb[32 * g : 32 * g + N_IN, m * 128 : (m + 1) * 128]),
                    mm(xT_sb[32 * g : 32 * g + N_IN, bs]),
                    start=True,
                    stop=True,
                    tile_position=(32 * g, 0),
                )
                ht = hpool.tile([128, NB], io_dt, tag=f"h{m}")
                if m % 2 == 0:
                    nc.scalar.activation(ht, ph, Relu, bias=b1_sb[:, m : m + 1])
                else:
                    nc.vector.tensor_scalar(
                        out=ht, in0=ph, scalar1=b1_sb[:, m : m + 1],
                        scalar2=0.0, op0=add_op, op1=max_op,
                    )
                hT[m] = ht
            return hT

        def mm23(c, hT):
            bs = slice(c * NB, (c + 1) * NB)
            # mm2: aT = relu(W21^T @ hT + b21)
            aT = []
            for m in range(MT2):
                pa = ps_a.tile([128, NB], f32)
                for k in range(KT2):
                    nc.tensor.matmul(
                        pa,
                        mm(w21_t[k][:, m * 128 : (m + 1) * 128]),
                        mm(hT[k]),
                        start=(k == 0),
                        stop=(k == KT2 - 1),
                    )
                at = apool.tile([128, NB], io_dt, tag=f"a{m}")
                if m % 2 == 0:
                    nc.scalar.activation(at, pa, Relu, bias=b21_sb[:, m : m + 1])
                else:
                    nc.vector.tensor_scalar(
                        out=at, in0=pa, scalar1=b21_sb[:, m : m + 1],
                        scalar2=0.0, op0=add_op, op1=max_op,
                    )
                aT.append(at)
            # mm3 + QP postprocess: out = clip(-t + bias2, lo2, hi2)
            po = ps_o.tile([N_CL, NB], f32)
            for k in range(KT3):
                nc.tensor.matmul(
                    po,
                    mm(w31_sb[:, k * N_CL : (k + 1) * N_CL]),
                    mm(aT[k]),
                    start=(k == 0),
                    stop=(k == KT3 - 1),
                )
            v = opool.tile([N_CL, NB], f32, tag="v")
            nc.scalar.activation(v, po, Ident, bias=post_sb[:, 0:1], scale=-1.0)
            nc.vector.tensor_scalar(
                out=v, in0=v, scalar1=post_sb[:, 1:2], scalar2=post_sb[:, 2:3],
                op0=max_op, op1=min_op,
            )
            nc.sync.dma_start(out=out_d[bs, :].rearrange("n c -> c n"), in_=v)

        # Software pipeline: mm1 runs LEAD chunks ahead of mm2/mm3 so the PE
        # has dense work while the 8MB W21 DMA streams in.
        LEAD = 2
        hts = {c: mm1(c) for c in range(min(LEAD, N_CHUNKS))}
        for c in range(N_CHUNKS):
            if c + LEAD < N_CHUNKS:
                hts[c + LEAD] = mm1(c + LEAD)
            mm23(c, hts.pop(c))

    nc.compile()
    return nc


def _get_nc():
    if MM_MODE not in _CACHE:
        _CACHE[MM_MODE] = _build(MM_MODE)
    return _CACHE[MM_MODE]


def kernel(**inputs):
    global last_results
    from concourse.bass_utils import run_bass_kernel_spmd

    f32 = np.float32
    x = np.asarray(inputs["x"], f32)
    W1 = np.asarray(inputs["W1"], f32)
    b1 = np.ascontiguousarray(np.asarray(inputs["b1"], f32))
    W21 = np.asarray(inputs["W21"], f32)
    b21 = np.ascontiguousarray(np.asarray(inputs["b21"], f32))
    W31 = np.asarray(inputs["W31"], f32)
    b31 = np.asarray(inputs["b31"], f32)
    om = np.asarray(inputs["output_mean"], f32)
    os_ = np.asarray(inputs["output_std"], f32)
    s0 = np.asarray(inputs["s0"], f32)[0]
    s1 = np.asarray(inputs["s1"], f32)[0]
    s2 = np.asarray(inputs["s2"], f32)[0]
    s3 = np.asarray(inputs["s3"], f32)[0]

    lo = np.array([-(1.0 + s3), -(1.0 + s1)], f32)
    hi = np.array([1.0 + s2, 1.0 + s0], f32)
    bias2 = -(b31 + 2.0 * om / os_)
    post = np.ascontiguousarray(
        np.stack([bias2, (lo - om) / os_, (hi - om) / os_], axis=1).astype(f32)
    )

    if MM_MODE == "bf16":
        import ml_dtypes

        conv = lambda a: np.ascontiguousarray(a.astype(ml_dtypes.bfloat16))
    else:
        conv = lambda a: np.ascontiguousarray(a)
    w1c, w21c, w31c = conv(W1), conv(W21), conv(W31)

    in_maps = []
    for c in range(N_CORES):
        xT = conv(x[c * B_SH : (c + 1) * B_SH].T)
        in_maps.append(
            {"xT": xT, "w1": w1c, "w21": w21c, "w31": w31c,
             "b1": b1, "b21": b21, "post": post}
        )

    nc = _get_nc()
    last_results = run_bass_kernel_spmd(
        nc, in_maps, list(range(N_CORES)), trace=TRACE
    )
    return np.concatenate(
        [last_results.results[c]["out"] for c in range(N_CORES)], axis=0
    ).astype(f32)



# revision 2
# speedup vs baseline: 1.2861x; 1.2861x over previous
"""BarrierNet forward on 8 Trainium2 NeuronCores (pure batch data-parallel).

Math actually needed (x32 / x0 branches of the reference are dead code):
    h   = relu(x @ W1 + b1)                       [B, 2048]
    a   = relu(h @ W21 + b21)                     [B, 1024]
    t   = a @ W31                                 [B, 2]    (bias folded below)
    out = clip(-t + bias2, lo2, hi2)              [B, 2]
with host-folded per-channel constants
    bias2 = -(b31 + 2*om/os),  lo2 = (lo-om)/os,  hi2 = (hi-om)/os
    lo = [-(1+s3), -(1+s1)],   hi = [1+s2, 1+s0]

Device dataflow keeps features on the partition dim (x^T -> h^T -> a^T ->
x31^T) so every weight matrix is used directly as the stationary lhsT.

mm1's K=8 is padded to a standard K=128 matmul by replicating x 16x along
the feature dim and host-prepping W1_rep[p, j] = W1[p % 8, j] / 16 — the
sum over 128 rows is 16 identical groups scaled back by 1/16. This keeps
every PE instruction in the same (128, 128) tile config; the previous
K=8 + tile_position packing forced PE tile-mode switches that stalled
adjacent mm2 matmuls by 2-3x.

The output stays feature-major on device ([2, B_SH], contiguous 2-row
DMA per chunk) and is transposed on host; a [B_SH, 2] device layout
needs a 512-descriptor scatter DMA per chunk that hammers the DMA queues
for the whole kernel and adds a ~38us tail after compute ends.
"""

import os

import numpy as np

B, N_IN, H1, H2, N_CL = 32768, 8, 2048, 1024, 2
N_CORES = 8
B_SH = B // N_CORES  # 4096 rows per core
NB = 512             # batch-chunk width (matmul free dim / PSUM bank)
N_CHUNKS = B_SH // NB
REP = 128 // N_IN    # x replication factor for the K=128 mm1
MT1 = H1 // 128      # 16 output tiles of mm1
KT2, MT2 = H1 // 128, H2 // 128  # 16 k-tiles, 8 m-tiles of mm2
KT3 = H2 // 128      # 8 k-tiles of mm3

MM_MODE = os.environ.get("BARRIER_MM_MODE", "fp32r")  # fp32r | bf16 | fp32
TRACE = bool(int(os.environ.get("BARRIER_TRACE", "0")))

_CACHE = {}
last_results = None  # BassKernelResults of the most recent run (for test.py)


def _build(mode):
    from contextlib import ExitStack

    import concourse.bass as bass
    import concourse.mybir as mybir
    import concourse.tile as tile
    from concourse import bacc

    f32 = mybir.dt.float32
    if mode == "bf16":
        io_dt = mybir.dt.bfloat16
    elif mode == "fp32r":
        io_dt = mybir.dt.float32r
    else:
        io_dt = f32

    nc = bacc.Bacc("TRN2", debug=False, num_devices=N_CORES)

    xT_d = nc.dram_tensor("xT", [128, B_SH], io_dt, kind="ExternalInput").ap()
    w1_d = nc.dram_tensor("w1", [128, H1], io_dt, kind="ExternalInput").ap()
    w21_d = nc.dram_tensor("w21", [H1, H2], io_dt, kind="ExternalInput").ap()
    w31_d = nc.dram_tensor("w31", [H2, N_CL], io_dt, kind="ExternalInput").ap()
    b1_d = nc.dram_tensor("b1", [H1], f32, kind="ExternalInput").ap()
    b21_d = nc.dram_tensor("b21", [H2], f32, kind="ExternalInput").ap()
    post_d = nc.dram_tensor("post", [N_CL, 3], f32, kind="ExternalInput").ap()
    out_d = nc.dram_tensor("out", [N_CL, B_SH], f32, kind="ExternalOutput").ap()

    Relu = mybir.ActivationFunctionType.Relu
    Ident = mybir.ActivationFunctionType.Identity
    add_op = mybir.AluOpType.add
    max_op = mybir.AluOpType.max
    min_op = mybir.AluOpType.min

    with tile.TileContext(nc) as tc, ExitStack() as ctx:
        const = ctx.enter_context(tc.tile_pool(name="const", bufs=1))
        wpool = ctx.enter_context(tc.tile_pool(name="w21", bufs=1))
        hpool = ctx.enter_context(tc.tile_pool(name="hT", bufs=3))
        apool = ctx.enter_context(tc.tile_pool(name="aT", bufs=1))
        opool = ctx.enter_context(tc.tile_pool(name="post", bufs=2))
        ps_h = ctx.enter_context(tc.tile_pool(name="ps_h", bufs=4, space="PSUM"))
        ps_a = ctx.enter_context(tc.tile_pool(name="ps_a", bufs=3, space="PSUM"))
        ps_o = ctx.enter_context(tc.tile_pool(name="ps_o", bufs=1, space="PSUM"))

        # Stationary weights / constants. DMA issue order is the critical
        # path: SP issues serially and nothing lands before ~12us of NEFF
        # startup. Put exactly what the first matmuls + evacs need first:
        # chunk-0 of xT, then w1 + b1; remaining chunks and constants; then
        # the 8MB W21.
        w1_sb = const.tile([128, H1], io_dt)
        xT_sb = const.tile([128, B_SH], io_dt)
        nc.sync.dma_start(out=xT_sb[:, 0:NB], in_=xT_d[:, 0:NB])
        nc.sync.dma_start(out=w1_sb, in_=w1_d)
        b1_sb = const.tile([128, MT1], f32)
        nc.sync.dma_start(out=b1_sb, in_=b1_d.rearrange("(k p) -> p k", p=128))
        for c in range(1, N_CHUNKS):
            nc.sync.dma_start(
                out=xT_sb[:, c * NB : (c + 1) * NB],
                in_=xT_d[:, c * NB : (c + 1) * NB],
            )
        b21_sb = const.tile([128, MT2], f32)
        nc.sync.dma_start(out=b21_sb, in_=b21_d.rearrange("(k p) -> p k", p=128))
        post_sb = const.tile([N_CL, 3], f32)
        nc.sync.dma_start(out=post_sb, in_=post_d)
        w31_sb = const.tile([128, KT3 * N_CL], io_dt)
        for k in range(KT3):
            nc.sync.dma_start(
                out=w31_sb[:, k * N_CL : (k + 1) * N_CL],
                in_=w31_d[k * 128 : (k + 1) * 128, :],
            )
        w21_t = []
        for k in range(KT2):
            t = wpool.tile([128, H2], io_dt, tag=f"w21_{k}")
            nc.sync.dma_start(out=t, in_=w21_d[k * 128 : (k + 1) * 128, :])
            w21_t.append(t)

        def mm1(c):
            # hT = relu(W1_rep^T @ xT_rep + b1), uniform K=128 matmuls.
            bs = slice(c * NB, (c + 1) * NB)
            hT = []
            for m in range(MT1):
                ph = ps_h.tile([128, NB], f32)
                nc.tensor.matmul(
                    ph,
                    w1_sb[:, m * 128 : (m + 1) * 128],
                    xT_sb[:, bs],
                    start=True,
                    stop=True,
                )
                ht = hpool.tile([128, NB], io_dt, tag=f"h{m}")
                if m % 2 == 0:
                    nc.scalar.activation(ht, ph, Relu, bias=b1_sb[:, m : m + 1])
                else:
                    nc.vector.tensor_scalar(
                        out=ht, in0=ph, scalar1=b1_sb[:, m : m + 1],
                        scalar2=0.0, op0=add_op, op1=max_op,
                    )
                hT.append(ht)
            return hT

        def mm23(c, hT):
            bs = slice(c * NB, (c + 1) * NB)
            # mm2: aT = relu(W21^T @ hT + b21)
            aT = []
            for m in range(MT2):
                pa = ps_a.tile([128, NB], f32)
                for k in range(KT2):
                    nc.tensor.matmul(
                        pa,
                        w21_t[k][:, m * 128 : (m + 1) * 128],
                        hT[k],
                        start=(k == 0),
                        stop=(k == KT2 - 1),
                    )
                at = apool.tile([128, NB], io_dt, tag=f"a{m}")
                if m % 2 == 0:
                    nc.scalar.activation(at, pa, Relu, bias=b21_sb[:, m : m + 1])
                else:
                    nc.vector.tensor_scalar(
                        out=at, in0=pa, scalar1=b21_sb[:, m : m + 1],
                        scalar2=0.0, op0=add_op, op1=max_op,
                    )
                aT.append(at)
            # mm3 + QP postprocess: out = clip(-t + bias2, lo2, hi2)
            po = ps_o.tile([N_CL, NB], f32)
            for k in range(KT3):
                nc.tensor.matmul(
                    po,
                    w31_sb[:, k * N_CL : (k + 1) * N_CL],
                    aT[k],
                    start=(k == 0),
                    stop=(k == KT3 - 1),
                )
            v = opool.tile([N_CL, NB], f32, tag="v")
            nc.scalar.activation(v, po, Ident, bias=post_sb[:, 0:1], scale=-1.0)
            nc.vector.tensor_scalar(
                out=v, in0=v, scalar1=post_sb[:, 1:2], scalar2=post_sb[:, 2:3],
                op0=max_op, op1=min_op,
            )
            nc.sync.dma_start(out=out_d[:, bs], in_=v)

        # Software pipeline: mm1 runs LEAD chunks ahead of mm2/mm3 so the PE
        # has dense work while the 8MB W21 DMA streams in.
        LEAD = 2
        hts = {c: mm1(c) for c in range(min(LEAD, N_CHUNKS))}
        for c in range(N_CHUNKS):
            if c + LEAD < N_CHUNKS:
                hts[c + LEAD] = mm1(c + LEAD)
            mm23(c, hts.pop(c))

    nc.compile()
    return nc


def _get_nc():
    if MM_MODE not in _CACHE:
        _CACHE[MM_MODE] = _build(MM_MODE)
    return _CACHE[MM_MODE]


def kernel(**inputs):
    global last_results
    from concourse.bass_utils import run_bass_kernel_spmd

    f32 = np.float32
    x = np.asarray(inputs["x"], f32)
    W1 = np.asarray(inputs["W1"], f32)
    b1 = np.ascontiguousarray(np.asarray(inputs["b1"], f32))
    W21 = np.asarray(inputs["W21"], f32)
    b21 = np.ascontiguousarray(np.asarray(inputs["b21"], f32))
    W31 = np.asarray(inputs["W31"], f32)
    b31 = np.asarray(inputs["b31"], f32)
    om = np.asarray(inputs["output_mean"], f32)
    os_ = np.asarray(inputs["output_std"], f32)
    s0 = np.asarray(inputs["s0"], f32)[0]
    s1 = np.asarray(inputs["s1"], f32)[0]
    s2 = np.asarray(inputs["s2"], f32)[0]
    s3 = np.asarray(inputs["s3"], f32)[0]

    lo = np.array([-(1.0 + s3), -(1.0 + s1)], f32)
    hi = np.array([1.0 + s2, 1.0 + s0], f32)
    bias2 = -(b31 + 2.0 * om / os_)
    post = np.ascontiguousarray(
        np.stack([bias2, (lo - om) / os_, (hi - om) / os_], axis=1).astype(f32)
    )

    if MM_MODE == "bf16":
        import ml_dtypes

        conv = lambda a: np.ascontiguousarray(a.astype(ml_dtypes.bfloat16))
    else:
        conv = lambda a: np.ascontiguousarray(a)
    # K=128 mm1 operands: W1 tiled REP times along features, scaled by
    # 1/REP (exact power of two); x replicated to match.
    w1_rep = np.tile(W1, (REP, 1)) * f32(1.0 / REP)
    w1c, w21c, w31c = conv(w1_rep), conv(W21), conv(W31)

    in_maps = []
    for c in range(N_CORES):
        xT = np.tile(x[c * B_SH : (c + 1) * B_SH].T, (REP, 1))
        in_maps.append(
            {"xT": conv(xT), "w1": w1c, "w21": w21c, "w31": w31c,
             "b1": b1, "b21": b21, "post": post}
        )

    nc = _get_nc()
    last_results = run_bass_kernel_spmd(
        nc, in_maps, list(range(N_CORES)), trace=TRACE
    )
    return np.ascontiguousarray(
        np.concatenate(
            [last_results.results[c]["out"].T for c in range(N_CORES)], axis=0
        ).astype(f32)
    )
